# revision 12
# baseline (speedup 1.0000x reference)
"""Trainium2 Bass kernel for nn_AttnInteractionLayer_2851858284689.

Math note: the reference's einsum ``'rfdh,rfoh->rfoh'`` contracts alpha over
its *softmax* axis, and softmax sums to one along that axis.  The attention
output therefore collapses exactly to ``vals``, and the module reduces to

    out = LayerNorm( leaky_relu( x @ (W_v.reshape(256,512) + W_r) ) )

Distribution: pure data parallel over the 4096*32 = 131072 (row, field)
tokens: 16384 tokens per NeuronCore, weights replicated.

Per-core schedule (16 blocks of 1024 tokens, 8 sub-tiles of 128 each):
  - PE:   bf16 matmuls y[128t, 512j] += xT[k,128t].T @ W[k,512j], fp32 PSUM
  - ACT:  batched Prelu (alpha=.01) [128, 4*512] PSUM -> bf16 SBUF; Sqrt of
          the 4-block stats batch; 4 of 8 normalizes per block as
          Identity(z*rstd + (-mean*rstd))
  - DVE:  bn_stats per sub-tile; manual even/odd halves combine (replaces
          bn_aggr): var = (cv_e+cv_o)/512 + ((m_e-m_o)/2)^2, batched over
          4 blocks; reciprocal; the other 4 normalizes as
          tensor_scalar(z*rstd - mean*rstd)
  - DMA:  whole x shard preloaded in 2 MB chunks; per-block output with
          8 KB-contiguous-per-partition rows (host pre-permutes token order)
"""

import numpy as np
import ml_dtypes

import concourse.bass as bass
import concourse.tile as tile
from concourse import bacc, mybir
from concourse.bass_utils import run_bass_kernel_spmd


def _ensure_ntff_hook():
    """This image lacks ``antenv.axon_hooks``; inject it (ctypes on
    libaxon_pjrt.so) so run_bass_kernel_spmd(trace=True) works."""
    try:
        from antenv.axon_hooks import get_axon_ntff_profile_hook  # noqa: F401
        return
    except ImportError:
        pass
    try:
        import contextlib
        import ctypes
        import sys
        import types

        lib = ctypes.CDLL("/opt/axon/libaxon_pjrt.so")
        if not hasattr(lib, "axon_start_nrt_profile"):
            return
        lib.axon_start_nrt_profile.argtypes = [
            ctypes.POINTER(ctypes.c_int64), ctypes.c_size_t]
        lib.axon_start_nrt_profile.restype = ctypes.c_int64
        lib.axon_stop_nrt_profile.argtypes = [ctypes.c_char_p]
        lib.axon_stop_nrt_profile.restype = ctypes.c_int64

        @contextlib.contextmanager
        def _hook(output_dir, device_ids):
            import jax
            jax.devices()
            if device_ids:
                ids = (ctypes.c_int64 * len(device_ids))(*device_ids)
                rc = lib.axon_start_nrt_profile(ids, len(device_ids))
            else:
                rc = lib.axon_start_nrt_profile(None, 0)
            if rc != 0:
                raise RuntimeError(f"axon_start_nrt_profile rc={rc}")
            try:
                yield
            finally:
                lib.axon_stop_nrt_profile(str(output_dir).encode())

        import antenv
        mod = types.ModuleType("antenv.axon_hooks")
        mod.get_axon_ntff_profile_hook = lambda: _hook
        mod.set_axon_ntff_profile_hook = lambda h: None
        sys.modules["antenv.axon_hooks"] = mod
        antenv.axon_hooks = mod
    except Exception:
        pass


_ensure_ntff_hook()

R, F, IN, OUT_TOT = 4096, 32, 256, 512
N_CORES = 8
TOKENS = R * F
TPC = TOKENS // N_CORES          # tokens per core: 16384
KC = IN // 128                   # contraction chunks: 2
BLK = 1024                       # token block
NBLK = TPC // BLK                # 16
GRP = 4                          # sub-tiles per PSUM group (4 banks)
SUB = BLK // 128                 # 8 sub-tiles per block
SB = 4                           # stats batch: blocks per sqrt/recip round
N_ACT_NORM = 4                   # sub-tiles normalized on ACT (rest on DVE)
XCHUNK = 1024                    # x preload chunk (columns) = 0.5 MB
EPS = 1e-5
NEG_SLOPE = 0.01
BF16 = mybir.dt.bfloat16
F32 = mybir.dt.float32

_compiled = {}


def _build_nc():
    nc = bacc.Bacc(None)
    xT = nc.declare_dram_parameter("xT", [KC, 128, TPC], BF16, isOutput=False)
    w = nc.declare_dram_parameter("w", [KC, 128, OUT_TOT], BF16, isOutput=False)
    y = nc.declare_dram_parameter("y", [TPC, OUT_TOT], BF16, isOutput=True)

    M = mybir.AluOpType.mult
    ADD = mybir.AluOpType.add
    SUBT = mybir.AluOpType.subtract

    with tile.TileContext(nc) as tc:
        with (
            tc.tile_pool(name="singles", bufs=1) as singles,
            tc.tile_pool(name="zpool", bufs=12) as zpool,
            tc.tile_pool(name="opool", bufs=4) as opool,
            tc.tile_pool(name="stats", bufs=2) as stats_pool,
            tc.tile_pool(name="small", bufs=2) as small_pool,
            tc.tile_pool(name="psum", bufs=2, space="PSUM") as psum,
        ):
            w_sb = singles.tile([128, KC, OUT_TOT], BF16)
            nc.sync.dma_start(out=w_sb, in_=w[:].rearrange("c k n -> k c n"))
            eps_sb = singles.tile([128, 1], F32)
            nc.vector.memset(eps_sb, EPS)

            # warm the ACT table set (Prelu/Sqrt/Identity share one) while
            # the first x chunk is still in flight
            warm = singles.tile([128, 2], F32)
            nc.vector.memset(warm, 1.0)
            nc.scalar.activation(
                warm[:, 1:2], warm[:, 0:1],
                mybir.ActivationFunctionType.Prelu, alpha=NEG_SLOPE)

            # whole x shard resident in SBUF; one tile per 1 MB chunk so
            # the first matmuls only depend on the first chunk's DMA
            widths = [256, 256, 512] + [XCHUNK] * ((TPC - 1024) // XCHUNK)
            starts = [sum(widths[:q]) for q in range(len(widths))]
            x_chunks = [singles.tile([128, KC, wd], BF16, name=f"xc{q}")
                        for q, wd in enumerate(widths)]
            issued = 0

            def issue_x(up_to_col):
                nonlocal issued
                while issued < len(widths) and starts[issued] < up_to_col:
                    q = issued
                    nc.sync.dma_start(
                        out=x_chunks[q],
                        in_=xT[:, :, starts[q]:starts[q] + widths[q]].rearrange(
                            "c k t -> k c t"),
                    )
                    issued += 1

            def x_slice(k, col):
                import bisect
                q = bisect.bisect_right(starts, col) - 1
                return x_chunks[q][:, k, col - starts[q]:col - starts[q] + 128]

            # per-stats-batch state carried across the SB-block round
            z_tiles = {}      # (block, grp) -> z tile
            st4 = None        # [128, SB*8, 6]
            rnd = {}          # round -> (r4, mr4, nmr4)

            def emit_norm_block(bb, n_act=N_ACT_NORM):
                gg = bb % SB
                r4, mr4, nmr4 = rnd[bb // SB]
                o = opool.tile([128, SUB, OUT_TOT], BF16)
                for i in range(SUB):
                    zs = z_tiles[(bb, i // GRP)][:, i % GRP, :]
                    idx = gg * SUB + i
                    if (i % 2 == 0 and i // 2 < n_act) or (i == 1 and n_act >= 5):
                        # ACT: z*r + (-m*r)
                        nc.scalar.activation(
                            o[:, i, :], zs,
                            mybir.ActivationFunctionType.Identity,
                            bias=nmr4[:, idx:idx + 1],
                            scale=r4[:, idx:idx + 1],
                        )
                    else:
                        # DVE: z*r - m*r
                        nc.vector.tensor_scalar(
                            o[:, i, :], zs,
                            r4[:, idx:idx + 1],
                            mr4[:, idx:idx + 1],
                            M, SUBT,
                        )
                del z_tiles[(bb, 0)], z_tiles[(bb, 1)]
                nc.sync.dma_start(
                    out=y[bb * BLK:(bb + 1) * BLK, :].rearrange(
                        "(p i) j -> p i j", p=128),
                    in_=o,
                )

            for b in range(NBLK):
                # prefetch x: chunk q covers blocks 2q, 2q+1; stay ~2 ahead
                issue_x((b + 3) * BLK)
                g = b % SB
                if g == 0:
                    st4 = stats_pool.tile([128, SB * SUB, 6], F32)

                for grp in range(2):
                    ps = psum.tile([128, GRP, OUT_TOT], F32)
                    for j in range(GRP):
                        i = grp * GRP + j
                        col = b * BLK + i * 128
                        nc.tensor.matmul(
                            ps[:, j, :], lhsT=x_slice(0, col),
                            rhs=w_sb[:, 0, :], start=True, stop=False,
                        )
                        nc.tensor.matmul(
                            ps[:, j, :], lhsT=x_slice(1, col),
                            rhs=w_sb[:, 1, :], start=False, stop=True,
                        )
                    z = zpool.tile([128, GRP, OUT_TOT], BF16)
                    nc.scalar.activation(
                        z, ps, mybir.ActivationFunctionType.Prelu,
                        alpha=NEG_SLOPE,
                    )
                    z_tiles[(b, grp)] = z
                    for j in range(GRP):
                        nc.vector.bn_stats(
                            st4[:, g * SUB + grp * GRP + j, :], z[:, j, :])

                if g == SB - 1:
                    # batched halves-combine over SB blocks: [128, 32]
                    n = SB * SUB
                    s = small_pool.tile([128, n], F32)
                    d = small_pool.tile([128, n], F32)
                    cv = small_pool.tile([128, n], F32)
                    v4 = small_pool.tile([128, n], F32)
                    std = small_pool.tile([128, n], F32)
                    r4 = small_pool.tile([128, n], F32)
                    mr4 = small_pool.tile([128, n], F32)
                    nmr4 = small_pool.tile([128, n], F32)
                    nc.vector.tensor_tensor(s, st4[:, :, 1], st4[:, :, 4], ADD)
                    nc.vector.tensor_tensor(d, st4[:, :, 1], st4[:, :, 4], SUBT)
                    nc.vector.tensor_tensor(cv, st4[:, :, 2], st4[:, :, 5], ADD)
                    nc.vector.tensor_tensor(d, d, d, M)
                    # v4 = 4*var = cv*(4/512) + d^2
                    nc.vector.scalar_tensor_tensor(
                        v4, cv, 4.0 / OUT_TOT, d, M, ADD)
                    # std = sqrt(0.25*v4 + eps)
                    nc.scalar.activation(
                        std, v4, mybir.ActivationFunctionType.Sqrt,
                        bias=eps_sb, scale=0.25,
                    )
                    nc.vector.reciprocal(r4, std)
                    # mr = mean*rstd = (s*0.5)*r ; nmr = -mr
                    nc.vector.scalar_tensor_tensor(mr4, s, 0.5, r4, M, M)
                    nc.vector.tensor_scalar_mul(nmr4, mr4, -1.0)
                    rnd[b // SB] = (r4, mr4, nmr4)

                # software pipeline: normalize block b-SB (its round's
                # stats chain was emitted at the end of block b-1 or
                # earlier), so norms never head-of-line-block the next
                # round's Prelu/bn_stats in the engine queues
                if b - SB >= 0:
                    emit_norm_block(b - SB, n_act=5)

            for bb in range(NBLK - SB, NBLK):
                emit_norm_block(bb, n_act=4)
    nc.finalize()
    return nc


def _get_nc():
    if "nc" not in _compiled:
        _compiled["nc"] = _build_nc()
    return _compiled["nc"]


def _in_maps(x, W_v, W_r):
    x = np.asarray(x, dtype=np.float32)
    W = (np.asarray(W_v, dtype=np.float32).reshape(IN, OUT_TOT)
         + np.asarray(W_r, dtype=np.float32))
    w_dev = np.ascontiguousarray(
        W.reshape(KC, 128, OUT_TOT).astype(ml_dtypes.bfloat16))

    xs = x.reshape(TOKENS, IN)
    in_maps = []
    for c in range(N_CORES):
        shard = xs[c * TPC:(c + 1) * TPC]                      # [TPC, IN]
        # device column order: col = b*1024 + i*128 + p holds host token
        # b*1024 + p*8 + i  (so output rows are contiguous per partition)
        perm = (shard.reshape(NBLK, 128, SUB, IN)
                .transpose(0, 2, 1, 3).reshape(TPC, IN))
        xT = np.ascontiguousarray(perm.T.astype(ml_dtypes.bfloat16))
        in_maps.append({"xT": xT.reshape(KC, 128, TPC), "w": w_dev})
    return in_maps


def _gather(res):
    out = np.concatenate([res.results[c]["y"] for c in range(N_CORES)], axis=0)
    return out.reshape(R, F, OUT_TOT).astype(np.float32)


def kernel(x, W_q, W_k, W_v, W_r, ln_gamma, ln_beta):
    nc = _get_nc()
    in_maps = _in_maps(x, W_v, W_r)
    res = run_bass_kernel_spmd(nc, in_maps, list(range(N_CORES)))
    out = _gather(res)

    gamma = np.asarray(ln_gamma, dtype=np.float32)
    beta = np.asarray(ln_beta, dtype=np.float32)
    if not (np.all(gamma == 1.0) and np.all(beta == 0.0)):
        out = out * gamma + beta
    return out.astype(np.float32)


# revision 13
# speedup vs baseline: 1.0657x; 1.0657x over previous
"""Trainium2 Bass kernel for nn_AttnInteractionLayer_2851858284689.

Math note: the reference's einsum ``'rfdh,rfoh->rfoh'`` contracts alpha over
its *softmax* axis, and softmax sums to one along that axis.  The attention
output therefore collapses exactly to ``vals``, and the module reduces to

    out = LayerNorm( leaky_relu( x @ (W_v.reshape(256,512) + W_r) ) )

Distribution: pure data parallel over the 4096*32 = 131072 (row, field)
tokens: 16384 tokens per NeuronCore, weights replicated.

Per-core schedule (16 blocks of 1024 tokens, 8 sub-tiles of 128 each):
  - PE:   bf16 matmuls y[128t, 512j] += xT[k,128t].T @ W[k,512j], fp32 PSUM
  - ACT:  batched Prelu (alpha=.01) [128, 4*512] PSUM -> bf16 SBUF; Sqrt of
          the 4-block stats batch; 4 of 8 normalizes per block as
          Identity(z*rstd + (-mean*rstd))
  - DVE:  bn_stats per sub-tile; manual even/odd halves combine (replaces
          bn_aggr): var = (cv_e+cv_o)/512 + ((m_e-m_o)/2)^2, batched over
          4 blocks; reciprocal; the other 4 normalizes as
          tensor_scalar(z*rstd - mean*rstd)
  - DMA:  whole x shard preloaded in 2 MB chunks; per-block output with
          8 KB-contiguous-per-partition rows (host pre-permutes token order)
"""

import numpy as np
import ml_dtypes

import concourse.bass as bass
import concourse.tile as tile
from concourse import bacc, mybir
from concourse.bass_utils import run_bass_kernel_spmd


def _ensure_ntff_hook():
    """This image lacks ``antenv.axon_hooks``; inject it (ctypes on
    libaxon_pjrt.so) so run_bass_kernel_spmd(trace=True) works."""
    try:
        from antenv.axon_hooks import get_axon_ntff_profile_hook  # noqa: F401
        return
    except ImportError:
        pass
    try:
        import contextlib
        import ctypes
        import sys
        import types

        lib = ctypes.CDLL("/opt/axon/libaxon_pjrt.so")
        if not hasattr(lib, "axon_start_nrt_profile"):
            return
        lib.axon_start_nrt_profile.argtypes = [
            ctypes.POINTER(ctypes.c_int64), ctypes.c_size_t]
        lib.axon_start_nrt_profile.restype = ctypes.c_int64
        lib.axon_stop_nrt_profile.argtypes = [ctypes.c_char_p]
        lib.axon_stop_nrt_profile.restype = ctypes.c_int64

        @contextlib.contextmanager
        def _hook(output_dir, device_ids):
            import jax
            jax.devices()
            if device_ids:
                ids = (ctypes.c_int64 * len(device_ids))(*device_ids)
                rc = lib.axon_start_nrt_profile(ids, len(device_ids))
            else:
                rc = lib.axon_start_nrt_profile(None, 0)
            if rc != 0:
                raise RuntimeError(f"axon_start_nrt_profile rc={rc}")
            try:
                yield
            finally:
                lib.axon_stop_nrt_profile(str(output_dir).encode())

        import antenv
        mod = types.ModuleType("antenv.axon_hooks")
        mod.get_axon_ntff_profile_hook = lambda: _hook
        mod.set_axon_ntff_profile_hook = lambda h: None
        sys.modules["antenv.axon_hooks"] = mod
        antenv.axon_hooks = mod
    except Exception:
        pass


_ensure_ntff_hook()

R, F, IN, OUT_TOT = 4096, 32, 256, 512
N_CORES = 8
TOKENS = R * F
TPC = TOKENS // N_CORES          # tokens per core: 16384
KC = IN // 128                   # contraction chunks: 2
BLK = 1024                       # token block
NBLK = TPC // BLK                # 16
GRP = 4                          # sub-tiles per PSUM group (4 banks)
SUB = BLK // 128                 # 8 sub-tiles per block
SB = 4                           # stats batch: blocks per sqrt/recip round
N_ACT_NORM = 4                   # sub-tiles normalized on ACT (rest on DVE)
XCHUNK = 1024                    # x preload chunk (columns) = 0.5 MB
EPS = 1e-5
NEG_SLOPE = 0.01
BF16 = mybir.dt.bfloat16
F32 = mybir.dt.float32

_compiled = {}


def _build_nc():
    nc = bacc.Bacc(None)
    xT = nc.declare_dram_parameter("xT", [KC, 128, TPC], BF16, isOutput=False)
    w = nc.declare_dram_parameter("w", [KC, 128, OUT_TOT], BF16, isOutput=False)
    y = nc.declare_dram_parameter("y", [TPC, OUT_TOT], BF16, isOutput=True)

    M = mybir.AluOpType.mult
    ADD = mybir.AluOpType.add
    SUBT = mybir.AluOpType.subtract

    with tile.TileContext(nc) as tc:
        with (
            tc.tile_pool(name="singles", bufs=1) as singles,
            tc.tile_pool(name="zpool", bufs=12) as zpool,
            tc.tile_pool(name="opool", bufs=4) as opool,
            tc.tile_pool(name="stats", bufs=2) as stats_pool,
            tc.tile_pool(name="small", bufs=2) as small_pool,
            tc.tile_pool(name="psum", bufs=2, space="PSUM") as psum,
        ):
            w_sb = singles.tile([128, KC, OUT_TOT], BF16)
            nc.sync.dma_start(out=w_sb, in_=w[:].rearrange("c k n -> k c n"))
            eps_sb = singles.tile([128, 1], F32)
            nc.vector.memset(eps_sb, EPS)

            # warm the ACT table set (Prelu/Sqrt/Identity share one) while
            # the first x chunk is still in flight
            warm = singles.tile([128, 2], F32)
            nc.vector.memset(warm, 1.0)
            nc.scalar.activation(
                warm[:, 1:2], warm[:, 0:1],
                mybir.ActivationFunctionType.Prelu, alpha=NEG_SLOPE)

            # whole x shard resident in SBUF; one tile per 1 MB chunk so
            # the first matmuls only depend on the first chunk's DMA
            widths = [256, 256, 512] + [XCHUNK] * ((TPC - 1024) // XCHUNK)
            starts = [sum(widths[:q]) for q in range(len(widths))]
            x_chunks = [singles.tile([128, KC, wd], BF16, name=f"xc{q}")
                        for q, wd in enumerate(widths)]
            issued = 0

            def issue_x(up_to_col):
                nonlocal issued
                while issued < len(widths) and starts[issued] < up_to_col:
                    q = issued
                    nc.sync.dma_start(
                        out=x_chunks[q],
                        in_=xT[:, :, starts[q]:starts[q] + widths[q]].rearrange(
                            "c k t -> k c t"),
                    )
                    issued += 1

            def x_slice(k, col):
                import bisect
                q = bisect.bisect_right(starts, col) - 1
                return x_chunks[q][:, k, col - starts[q]:col - starts[q] + 128]

            # per-stats-batch state carried across the SB-block round
            z_tiles = {}      # (block, grp) -> z tile
            st4 = None        # [128, SB*8, 6]
            rnd = {}          # round -> (r4, mr4, nmr4)

            def emit_norm_block(bb, n_act=N_ACT_NORM):
                gg = bb % SB
                r4, mr4, nmr4 = rnd[bb // SB]
                o = opool.tile([128, SUB, OUT_TOT], BF16)
                for i in range(SUB):
                    zs = z_tiles[(bb, i // GRP)][:, i % GRP, :]
                    idx = gg * SUB + i
                    if (i % 2 == 0 and i // 2 < n_act) or (i == 1 and n_act >= 5):
                        # ACT: z*r + (-m*r)
                        nc.scalar.activation(
                            o[:, i, :], zs,
                            mybir.ActivationFunctionType.Identity,
                            bias=nmr4[:, idx:idx + 1],
                            scale=r4[:, idx:idx + 1],
                        )
                    else:
                        # DVE: z*r - m*r
                        nc.vector.tensor_scalar(
                            o[:, i, :], zs,
                            r4[:, idx:idx + 1],
                            mr4[:, idx:idx + 1],
                            M, SUBT,
                        )
                del z_tiles[(bb, 0)], z_tiles[(bb, 1)]
                nc.sync.dma_start(
                    out=y[bb * BLK:(bb + 1) * BLK, :].rearrange(
                        "(p i) j -> p i j", p=128),
                    in_=o,
                )

            for b in range(NBLK):
                # prefetch x: chunk q covers blocks 2q, 2q+1; stay ~2 ahead
                issue_x((b + 3) * BLK)
                g = b % SB
                if g == 0:
                    st4 = stats_pool.tile([128, SB * SUB, 6], F32)

                for grp in range(2):
                    ps = psum.tile([128, GRP, OUT_TOT], F32)
                    for j in range(GRP):
                        i = grp * GRP + j
                        col = b * BLK + i * 128
                        nc.tensor.matmul(
                            ps[:, j, :], lhsT=x_slice(0, col),
                            rhs=w_sb[:, 0, :], start=True, stop=False,
                        )
                        nc.tensor.matmul(
                            ps[:, j, :], lhsT=x_slice(1, col),
                            rhs=w_sb[:, 1, :], start=False, stop=True,
                        )
                    z = zpool.tile([128, GRP, OUT_TOT], BF16)
                    nc.scalar.activation(
                        z, ps, mybir.ActivationFunctionType.Prelu,
                        alpha=NEG_SLOPE,
                    )
                    z_tiles[(b, grp)] = z
                    for j in range(GRP):
                        nc.vector.bn_stats(
                            st4[:, g * SUB + grp * GRP + j, :], z[:, j, :])

                if g == SB - 1:
                    # batched halves-combine over SB blocks: [128, 32]
                    n = SB * SUB
                    s = small_pool.tile([128, n], F32)
                    d = small_pool.tile([128, n], F32)
                    cv = small_pool.tile([128, n], F32)
                    v4 = small_pool.tile([128, n], F32)
                    std = small_pool.tile([128, n], F32)
                    r4 = small_pool.tile([128, n], F32)
                    mr4 = small_pool.tile([128, n], F32)
                    nmr4 = small_pool.tile([128, n], F32)
                    nc.vector.tensor_tensor(s, st4[:, :, 1], st4[:, :, 4], ADD)
                    nc.vector.tensor_tensor(d, st4[:, :, 1], st4[:, :, 4], SUBT)
                    nc.vector.tensor_tensor(cv, st4[:, :, 2], st4[:, :, 5], ADD)
                    nc.vector.tensor_tensor(d, d, d, M)
                    # v4 = 4*var = cv*(4/512) + d^2
                    nc.vector.scalar_tensor_tensor(
                        v4, cv, 4.0 / OUT_TOT, d, M, ADD)
                    # std = sqrt(0.25*v4 + eps)
                    nc.scalar.activation(
                        std, v4, mybir.ActivationFunctionType.Sqrt,
                        bias=eps_sb, scale=0.25,
                    )
                    nc.vector.reciprocal(r4, std)
                    # mr = mean*rstd = (s*0.5)*r ; nmr = -mr
                    nc.vector.scalar_tensor_tensor(mr4, s, 0.5, r4, M, M)
                    nc.vector.tensor_scalar_mul(nmr4, mr4, -1.0)
                    rnd[b // SB] = (r4, mr4, nmr4)

                # software pipeline: normalize block b-SB (its round's
                # stats chain was emitted at the end of block b-1 or
                # earlier), so norms never head-of-line-block the next
                # round's Prelu/bn_stats in the engine queues
                if b - SB >= 0:
                    emit_norm_block(b - SB)

            for bb in range(NBLK - SB, NBLK):
                emit_norm_block(bb, n_act=4)
    nc.finalize()
    return nc


def _get_nc():
    if "nc" not in _compiled:
        _compiled["nc"] = _build_nc()
    return _compiled["nc"]


def _in_maps(x, W_v, W_r):
    x = np.asarray(x, dtype=np.float32)
    W = (np.asarray(W_v, dtype=np.float32).reshape(IN, OUT_TOT)
         + np.asarray(W_r, dtype=np.float32))
    w_dev = np.ascontiguousarray(
        W.reshape(KC, 128, OUT_TOT).astype(ml_dtypes.bfloat16))

    xs = x.reshape(TOKENS, IN)
    in_maps = []
    for c in range(N_CORES):
        shard = xs[c * TPC:(c + 1) * TPC]                      # [TPC, IN]
        # device column order: col = b*1024 + i*128 + p holds host token
        # b*1024 + p*8 + i  (so output rows are contiguous per partition)
        perm = (shard.reshape(NBLK, 128, SUB, IN)
                .transpose(0, 2, 1, 3).reshape(TPC, IN))
        xT = np.ascontiguousarray(perm.T.astype(ml_dtypes.bfloat16))
        in_maps.append({"xT": xT.reshape(KC, 128, TPC), "w": w_dev})
    return in_maps


def _gather(res):
    out = np.concatenate([res.results[c]["y"] for c in range(N_CORES)], axis=0)
    return out.reshape(R, F, OUT_TOT).astype(np.float32)


def kernel(x, W_q, W_k, W_v, W_r, ln_gamma, ln_beta):
    nc = _get_nc()
    in_maps = _in_maps(x, W_v, W_r)
    res = run_bass_kernel_spmd(nc, in_maps, list(range(N_CORES)))
    out = _gather(res)

    gamma = np.asarray(ln_gamma, dtype=np.float32)
    beta = np.asarray(ln_beta, dtype=np.float32)
    if not (np.all(gamma == 1.0) and np.all(beta == 0.0)):
        out = out * gamma + beta
    return out.astype(np.float32)
